# revision 1
# baseline (speedup 1.0000x reference)
"""Trainium2 Bass kernel for nn_DynamicNTKLayer.

Reference math (B=4, N=4096, D=1024, H=16, hd=64):
    phi      = x @ fm_w.T + fm_b                          (B, N, D)
    kernel   = einsum('bid,bjd->bij', phi, phi) * 0.5     (B, N, N)
    attended = MHA(x)   # attention over dim 0 (L=B), batched over dim 1
    out      = x + kernel @ attended

Key algebraic restructure: kernel @ attended = phi @ (phi.T @ attended), so
the (N,N) kernel matrix is never materialized.  With zero biases (the case
setup_inputs generates) this further reassociates to
    P2[b] = 0.5 * phi[b].T @ attn_out[b]                  (D, D)
    out   = x + (phi @ P2) @ out_w.T
so no matmul is ever replicated across cores.

Sharding: split N across the 8 cores (the MHA attends over dim 0, so it is
fully local under N-sharding).  Two SPMD launches with a host all-reduce of
the per-core partial P2 (or M, in the general-bias path) between them.

All matmuls run as float32r (TRN2 full-rate fp32 mode, ~1.3e-4 rel err).
PSUM->SBUF evictions run on the Scalar (ACT) engine, which is otherwise
idle; the Vector engine is reserved for the attention arithmetic.
"""

import sys
from contextlib import ExitStack

import numpy as np

sys.path.insert(0, "/opt/trn_rl_repo")

import concourse.bass as bass
import concourse.tile as tile
from concourse import bacc, mybir
from concourse.bass_utils import run_bass_kernel_spmd
from concourse.masks import make_identity

dt = mybir.dt
Alu = mybir.AluOpType
Axis = mybir.AxisListType

P = 128
B = 4
N_FULL = 4096
D = 1024
H = 16
HD = 64
NCORES = 8
ALPHA = 0.5
SCALE = 1.0 / 8.0  # 1/sqrt(hd)


def build_launch1(n_loc: int, with_bias: bool):
    if not with_bias:
        return _build_launch1_fast(n_loc)
    return _build_launch1_general(n_loc)


def _build_launch1_fast(n_loc: int):
    """Fast path (zero biases): fused transpose+qkv+attention pipeline.

    qkv never round-trips DRAM (evicted straight into attention tiles);
    phi stays SBUF-resident into the P2 reduction; xT goes to DRAM once
    and is re-streamed for the phi/phiT phase.
    """
    T = B * n_loc
    NT = T // P
    NN = n_loc // P
    DT = D // P

    nc = bacc.Bacc("TRN2", target_bir_lowering=False, debug=False,
                   num_devices=NCORES)

    x = nc.dram_tensor("x", [B, n_loc, D], dt.float32, kind="ExternalInput").ap()
    fm_wT = nc.dram_tensor("fm_wT", [D, D], dt.float32r, kind="ExternalInput").ap()
    fm_b = nc.dram_tensor("fm_b", [1, D], dt.float32r, kind="ExternalInput").ap()
    wqkvT = nc.dram_tensor("wqkvT", [D, 3 * D], dt.float32r, kind="ExternalInput").ap()
    qkv_b = nc.dram_tensor("qkv_b", [1, 3 * D], dt.float32r, kind="ExternalInput").ap()
    out_wT = nc.dram_tensor("out_wT", [D, D], dt.float32r, kind="ExternalInput").ap()
    out_b = nc.dram_tensor("out_b", [1, D], dt.float32r, kind="ExternalInput").ap()

    phiT_out = nc.dram_tensor("phiT_out", [D, T], dt.float32r, kind="ExternalOutput").ap()
    red_part = nc.dram_tensor("red_part", [B, D, D], dt.float32, kind="ExternalOutput").ap()

    xT_d = nc.dram_tensor("xT_d", [D, T], dt.float32r).ap()
    attn_d = nc.dram_tensor("attn_d", [T, D], dt.float32r).ap()

    xf = x.rearrange("b n d -> (b n) d")

    with tile.TileContext(nc) as tc, ExitStack() as ctx:
        const = ctx.enter_context(tc.tile_pool(name="const", bufs=1))
        ident = const.tile([P, P], dt.float32)
        make_identity(nc, ident[:])
        ident_r = const.tile([P, P], dt.float32r, tag="ident_r")
        nc.vector.tensor_copy(ident_r[:], ident[:])

        # ---- Ph1: fused transpose + qkv + attention, per n-slice ----------
        with tc.tile_pool(name="xin", bufs=2) as xin_pool, \
             tc.tile_pool(name="xts", bufs=12) as xts_pool, \
             tc.tile_pool(name="wq", bufs=DT) as w_pool, \
             tc.tile_pool(name="tp_ps", bufs=2, space="PSUM") as tp_psum, \
             tc.tile_pool(name="qkv_ps", bufs=6, space="PSUM") as qkv_psum, \
             tc.tile_pool(name="qkvt", bufs=16) as qkv_pool, \
             tc.tile_pool(name="sm", bufs=2) as sm_pool, \
             tc.tile_pool(name="tt", bufs=2) as tt_pool, \
             tc.tile_pool(name="acc", bufs=4) as acc_pool:
            wq = []
            for dtl in range(DT):
                wt = w_pool.tile([P, 3 * D], dt.float32r, tag="wq", name="wq")
                # split across both DMA paths so the first token tiles don't
                # stall behind the full 12MB weight transfer
                eng = nc.gpsimd if dtl % 2 else nc.sync
                eng.dma_start(wt[:], wqkvT[dtl * P:(dtl + 1) * P, :])
                wq.append(wt)

            for nt in range(NN):
                q = []; k = []; v = []
                for bb in range(B):
                    qt = qkv_pool.tile([P, D], dt.float32r, tag="qkvt", name="qkvt")
                    kt = qkv_pool.tile([P, D], dt.float32r, tag="qkvt", name="qkvt")
                    vt = qkv_pool.tile([P, D], dt.float32r, tag="qkvt", name="qkvt")
                    q.append(qt); k.append(kt); v.append(vt)

                for bb in range(B):
                    t = bb * NN + nt
                    # transpose this token tile into 8 xT blocks
                    xin = xin_pool.tile([P, D], dt.float32, tag="xin")
                    nc.sync.dma_start(xin[:], xf[t * P:(t + 1) * P, :])
                    xts = []
                    for dtl in range(DT):
                        ps = tp_psum.tile([P, P], dt.float32, tag="tp")
                        nc.tensor.transpose(ps[:], xin[:, dtl * P:(dtl + 1) * P],
                                            ident[:])
                        xt_ = xts_pool.tile([P, P], dt.float32r, tag="xts",
                                            name="xts")
                        nc.scalar.copy(xt_[:], ps[:])
                        nc.sync.dma_start(
                            xT_d[dtl * P:(dtl + 1) * P, t * P:(t + 1) * P],
                            xt_[:])
                        xts.append(xt_)
                    # qkv matmuls straight into attention tiles
                    pss = [qkv_psum.tile([P, 512], dt.float32, tag="qkvps",
                                         name="qkvps") for _ in range(6)]
                    for dtl in range(DT):
                        for s in range(6):
                            nc.tensor.matmul(pss[s][:], xts[dtl][:],
                                             wq[dtl][:, s * 512:(s + 1) * 512],
                                             start=(dtl == 0),
                                             stop=(dtl == DT - 1))
                    dest = [q[bb], q[bb], k[bb], k[bb], v[bb], v[bb]]
                    for s in range(6):
                        nc.scalar.copy(dest[s][:, (s % 2) * 512:(s % 2) * 512 + 512],
                                       pss[s][:])

                # attention for this n-slice (DVE/ACT)
                S = sm_pool.tile([P, B, H, B], dt.float32, tag="S")
                pairs = sorted(((l, m) for l in range(B) for m in range(B)),
                               key=lambda lm: (max(lm), lm))
                for l, m in pairs:
                    prod = tt_pool.tile([P, D], dt.float32, tag="prod")
                    nc.vector.tensor_tensor(prod[:], q[l][:], k[m][:],
                                            Alu.mult)
                    nc.vector.tensor_reduce(
                        S[:, l, :, m],
                        prod[:].rearrange("p (h d) -> p h d", d=HD),
                        Axis.X, Alu.add)
                S2 = S[:].rearrange("p l h m -> p (l h) m")
                nc.vector.tensor_scalar_mul(S2, S2, SCALE)
                mx = sm_pool.tile([P, B * H], dt.float32, tag="mx")
                nc.vector.tensor_reduce(mx[:], S2, Axis.X, Alu.max)
                E = sm_pool.tile([P, B, H, B], dt.float32, tag="E")
                E2 = E[:].rearrange("p l h m -> p (l h) m")
                nc.vector.tensor_tensor(
                    S2, S2, mx[:, :, None].to_broadcast([P, B * H, B]),
                    Alu.subtract)
                nc.scalar.activation(E2, S2, mybir.ActivationFunctionType.Exp)
                den = sm_pool.tile([P, B * H], dt.float32, tag="den")
                nc.vector.tensor_reduce(den[:], E2, Axis.X, Alu.add)
                rec = sm_pool.tile([P, B * H], dt.float32, tag="rec")
                nc.vector.reciprocal(rec[:], den[:])
                A = sm_pool.tile([P, B, H, B], dt.float32, tag="A")
                A2 = A[:].rearrange("p l h m -> p (l h) m")
                nc.vector.tensor_tensor(
                    A2, E2, rec[:, :, None].to_broadcast([P, B * H, B]),
                    Alu.mult)
                for l in range(B):
                    acc = acc_pool.tile([P, D], dt.float32r, tag="acc")
                    nc.vector.tensor_tensor(
                        acc[:].rearrange("p (h d) -> p h d", d=HD),
                        v[0][:].rearrange("p (h d) -> p h d", d=HD),
                        A[:, l, :, 0, None].to_broadcast([P, H, HD]),
                        Alu.mult)
                    for m in range(1, B):
                        tmp = tt_pool.tile([P, D], dt.float32, tag="prod")
                        nc.vector.tensor_tensor(
                            tmp[:].rearrange("p (h d) -> p h d", d=HD),
                            v[m][:].rearrange("p (h d) -> p h d", d=HD),
                            A[:, l, :, m, None].to_broadcast([P, H, HD]),
                            Alu.mult)
                        nc.vector.tensor_tensor(acc[:], acc[:], tmp[:],
                                                Alu.add)
                    row = l * n_loc + nt * P
                    nc.sync.dma_start(attn_d[row:row + P, :], acc[:])

        # ---- Ph2: phi (SBUF-resident) + phiT (to DRAM) ---------------------
        with tc.tile_pool(name="phi_sb", bufs=NT) as phi_pool:
            phi_sb = [phi_pool.tile([P, D], dt.float32r, tag="phi", name="phi")
                      for _ in range(NT)]
            with tc.tile_pool(name="xts2", bufs=16) as xts2_pool, \
                 tc.tile_pool(name="fmw", bufs=DT) as fm_pool, \
                 tc.tile_pool(name="phi_ps", bufs=3, space="PSUM") as phi_psum, \
                 tc.tile_pool(name="phiT_ps", bufs=4, space="PSUM") as phiT_psum, \
                 tc.tile_pool(name="phiT_ev", bufs=6) as phiT_ev:
                fmw = []
                for dtl in range(DT):
                    wt = fm_pool.tile([P, D], dt.float32r, tag="fmw", name="fmw")
                    nc.gpsimd.dma_start(wt[:], fm_wT[dtl * P:(dtl + 1) * P, :])
                    fmw.append(wt)

                # phi: token-major, kept in SBUF
                for t in range(NT):
                    xts = []
                    for dtl in range(DT):
                        xt_ = xts2_pool.tile([P, P], dt.float32r, tag="xts2",
                                             name="xts2")
                        nc.sync.dma_start(
                            xt_[:],
                            xT_d[dtl * P:(dtl + 1) * P, t * P:(t + 1) * P])
                        xts.append(xt_)
                    for s in range(2):
                        ps = phi_psum.tile([P, 512], dt.float32, tag="phips")
                        for dtl in range(DT):
                            nc.tensor.matmul(
                                ps[:], xts[dtl][:],
                                fmw[dtl][:, s * 512:(s + 1) * 512],
                                start=(dtl == 0), stop=(dtl == DT - 1))
                        nc.scalar.copy(phi_sb[t][:, s * 512:(s + 1) * 512],
                                       ps[:])

                # phiT: PE-transpose the resident phi tiles (no extra
                # matmul chain, no xT re-read)
                for t in range(NT):
                    for dtl in range(DT):
                        ps = phiT_psum.tile([P, P], dt.float32r, tag="phiTtp")
                        nc.tensor.transpose(
                            ps[:], phi_sb[t][:, dtl * P:(dtl + 1) * P],
                            ident_r[:])
                        ev = phiT_ev.tile([P, P], dt.float32r, tag="phiTev")
                        nc.scalar.copy(ev[:], ps[:])
                        nc.sync.dma_start(
                            phiT_out[dtl * P:(dtl + 1) * P,
                                     t * P:(t + 1) * P], ev[:])

            # ---- Ph4: red = P2 = 0.5 * phi^T @ attn  (dphi, din) -----------
            with tc.tile_pool(name="chunks", bufs=2 * NN + 2) as ch_pool, \
                 tc.tile_pool(name="p2ps", bufs=2, space="PSUM") as p2_psum, \
                 tc.tile_pool(name="mev", bufs=4) as mev_pool:
                for bb in range(B):
                    ac = []
                    for c in range(NN):
                        row = bb * n_loc + c * P
                        a_t = ch_pool.tile([P, D], dt.float32r, tag="ach",
                                           name="ach")
                        nc.gpsimd.dma_start(a_t[:], attn_d[row:row + P, :])
                        ac.append(a_t)
                    for dtl in range(DT):
                        pps = p2_psum.tile([P, D], dt.float32, tag="p2ps",
                                           name="p2ps")
                        for c in range(NN):
                            pc = phi_sb[bb * NN + c]
                            for s in range(2):
                                nc.tensor.matmul(
                                    pps[:, s * 512:(s + 1) * 512],
                                    pc[:, dtl * P:(dtl + 1) * P],
                                    ac[c][:, s * 512:(s + 1) * 512],
                                    start=(c == 0), stop=(c == NN - 1))
                        ev = mev_pool.tile([P, D], dt.float32, tag="mev")
                        nc.scalar.mul(ev[:], pps[:], ALPHA)
                        nc.sync.dma_start(
                            red_part[bb, dtl * P:(dtl + 1) * P, :], ev[:])

    nc.compile()
    return nc


def _build_launch1_general(n_loc: int):
    with_bias = True
    """Per-core program: x slice + weights -> phiT + partial reduction.

    with_bias=False (fast path): red_part = 0.5 * phi^T @ attn_out  (P2)
    with_bias=True  (general):   red_part = M = 0.5*(phi^T @ attended)
    """
    T = B * n_loc            # local token count (b-major flattening)
    NT = T // P              # token tiles
    NN = n_loc // P          # n tiles (attention batches 128 tokens over n)
    DT = D // P              # 8 partition tiles of D

    nc = bacc.Bacc("TRN2", target_bir_lowering=False, debug=False,
                   num_devices=NCORES)

    x = nc.dram_tensor("x", [B, n_loc, D], dt.float32, kind="ExternalInput").ap()
    fm_wT = nc.dram_tensor("fm_wT", [D, D], dt.float32r, kind="ExternalInput").ap()
    fm_b = nc.dram_tensor("fm_b", [1, D], dt.float32r, kind="ExternalInput").ap()
    wqkvT = nc.dram_tensor("wqkvT", [D, 3 * D], dt.float32r, kind="ExternalInput").ap()
    qkv_b = nc.dram_tensor("qkv_b", [1, 3 * D], dt.float32r, kind="ExternalInput").ap()
    out_wT = nc.dram_tensor("out_wT", [D, D], dt.float32r, kind="ExternalInput").ap()
    out_b = nc.dram_tensor("out_b", [1, D], dt.float32r, kind="ExternalInput").ap()

    phiT_out = nc.dram_tensor("phiT_out", [D, T], dt.float32r, kind="ExternalOutput").ap()
    red_part = nc.dram_tensor("red_part", [B, D, D], dt.float32, kind="ExternalOutput").ap()

    qkv_d = nc.dram_tensor("qkv_d", [T, 3 * D], dt.float32r).ap()
    attn_d = nc.dram_tensor("attn_d", [T, D], dt.float32r).ap()
    phi_d = nc.dram_tensor("phi_d", [T, D], dt.float32r).ap()

    xf = x.rearrange("b n d -> (b n) d")

    with tile.TileContext(nc) as tc, ExitStack() as ctx:
        const = ctx.enter_context(tc.tile_pool(name="const", bufs=1))
        ident = const.tile([P, P], dt.float32)
        make_identity(nc, ident[:])
        if with_bias:
            ones_f = const.tile([P, 512], dt.float32, tag="ones_f")
            nc.vector.memset(ones_f[:], 1.0)
            ones_r = const.tile([1, 512], dt.float32r, tag="ones_r")
            nc.vector.tensor_copy(ones_r[:], ones_f[:1, :])
            ones_c = const.tile([P, 1], dt.float32r, tag="ones_c")
            nc.vector.tensor_copy(ones_c[:], ones_f[:, :1])

        # xT lives through Ph0..Ph2/3, released before Ph4
        with tc.tile_pool(name="xT", bufs=DT) as xT_pool:
            xT = [xT_pool.tile([P, T], dt.float32r, tag="xT", name="xT")
                  for _ in range(DT)]

            # ---- Ph0: transpose x into xT ----------------------------------
            with tc.tile_pool(name="xin", bufs=3) as xin_pool, \
                 tc.tile_pool(name="tp_ps", bufs=4, space="PSUM") as tp_psum:
                for t in range(NT):
                    xin = xin_pool.tile([P, D], dt.float32, tag="xin")
                    nc.sync.dma_start(xin[:], xf[t * P:(t + 1) * P, :])
                    for dtl in range(DT):
                        ps = tp_psum.tile([P, P], dt.float32, tag="tp")
                        nc.tensor.transpose(ps[:], xin[:, dtl * P:(dtl + 1) * P],
                                            ident[:])
                        nc.scalar.copy(xT[dtl][:, t * P:(t + 1) * P], ps[:])

            # ---- Ph1: qkv = x @ Wqkv.T (+ b)  -> qkv_d ---------------------
            with tc.tile_pool(name="wq", bufs=DT) as w_pool, \
                 tc.tile_pool(name="qb", bufs=1) as qb_pool, \
                 tc.tile_pool(name="qkv_ps", bufs=8, space="PSUM") as qkv_psum, \
                 tc.tile_pool(name="qkv_ev", bufs=4) as qkv_ev:
                wq = []
                for dtl in range(DT):
                    wt = w_pool.tile([P, 3 * D], dt.float32r, tag="wq", name="wq")
                    nc.sync.dma_start(wt[:], wqkvT[dtl * P:(dtl + 1) * P, :])
                    wq.append(wt)
                if with_bias:
                    qb = qb_pool.tile([1, 3 * D], dt.float32r)
                    nc.sync.dma_start(qb[:], qkv_b[:])

                # n-major emission order so attention tiles unblock early
                for nt in range(NN):
                    for bb in range(B):
                        t = bb * NN + nt
                        pss = [qkv_psum.tile([P, 512], dt.float32, tag="qkvps",
                                             name="qkvps") for _ in range(6)]
                        for dtl in range(DT):
                            lhsT = xT[dtl][:, t * P:(t + 1) * P]
                            for s in range(6):
                                nc.tensor.matmul(pss[s][:], lhsT,
                                                 wq[dtl][:, s * 512:(s + 1) * 512],
                                                 start=(dtl == 0),
                                                 stop=(not with_bias and dtl == DT - 1))
                        for s in range(6):
                            if with_bias:
                                nc.tensor.matmul(pss[s][:], ones_r[:, :P],
                                                 qb[:, s * 512:(s + 1) * 512],
                                                 start=False, stop=True)
                            ev = qkv_ev.tile([P, 512], dt.float32r, tag="qkvev")
                            nc.scalar.copy(ev[:], pss[s][:])
                            nc.sync.dma_start(
                                qkv_d[t * P:(t + 1) * P, s * 512:(s + 1) * 512],
                                ev[:])

            # ---- Ph2+Ph3 interleaved: attention (DVE) overlaps phi (PE) ----
            with tc.tile_pool(name="fmw", bufs=DT) as fm_pool, \
                 tc.tile_pool(name="fmb", bufs=1) as fmb_pool, \
                 tc.tile_pool(name="phi_ps", bufs=4, space="PSUM") as phi_psum, \
                 tc.tile_pool(name="phi_ev", bufs=4) as phi_ev, \
                 tc.tile_pool(name="qkvt", bufs=3 * B) as qkv_pool, \
                 tc.tile_pool(name="sm", bufs=2) as sm_pool, \
                 tc.tile_pool(name="tt", bufs=2) as tt_pool, \
                 tc.tile_pool(name="acc", bufs=4) as acc_pool:
                fmw = []
                for dtl in range(DT):
                    wt = fm_pool.tile([P, D], dt.float32r, tag="fmw", name="fmw")
                    nc.sync.dma_start(wt[:], fm_wT[dtl * P:(dtl + 1) * P, :])
                    fmw.append(wt)
                if with_bias:
                    fmb = fmb_pool.tile([1, D], dt.float32r)
                    nc.sync.dma_start(fmb[:], fm_b[:])

                for nt in range(NN):
                    # -- attention for n-slice nt (DVE/ACT only) --
                    q = []; k = []; v = []
                    for bb in range(B):
                        row = bb * n_loc + nt * P
                        qt = qkv_pool.tile([P, D], dt.float32r, tag="qkvt",
                                           name="qkvt")
                        kt = qkv_pool.tile([P, D], dt.float32r, tag="qkvt",
                                           name="qkvt")
                        vt = qkv_pool.tile([P, D], dt.float32r, tag="qkvt",
                                           name="qkvt")
                        nc.sync.dma_start(qt[:], qkv_d[row:row + P, 0:D])
                        nc.sync.dma_start(kt[:], qkv_d[row:row + P, D:2 * D])
                        nc.sync.dma_start(vt[:], qkv_d[row:row + P, 2 * D:3 * D])
                        q.append(qt); k.append(kt); v.append(vt)

                    # scores S[p, l, h, m] = sum_d q[l]*k[m]
                    S = sm_pool.tile([P, B, H, B], dt.float32, tag="S")
                    for l in range(B):
                        for m in range(B):
                            prod = tt_pool.tile([P, D], dt.float32, tag="prod")
                            nc.vector.tensor_tensor(prod[:], q[l][:], k[m][:],
                                                    Alu.mult)
                            nc.vector.tensor_reduce(
                                S[:, l, :, m],
                                prod[:].rearrange("p (h d) -> p h d", d=HD),
                                Axis.X, Alu.add)
                    S2 = S[:].rearrange("p l h m -> p (l h) m")
                    nc.vector.tensor_scalar_mul(S2, S2, SCALE)
                    mx = sm_pool.tile([P, B * H], dt.float32, tag="mx")
                    nc.vector.tensor_reduce(mx[:], S2, Axis.X, Alu.max)
                    E = sm_pool.tile([P, B, H, B], dt.float32, tag="E")
                    E2 = E[:].rearrange("p l h m -> p (l h) m")
                    nc.vector.tensor_tensor(
                        S2, S2, mx[:, :, None].to_broadcast([P, B * H, B]),
                        Alu.subtract)
                    nc.scalar.activation(E2, S2,
                                         mybir.ActivationFunctionType.Exp)
                    den = sm_pool.tile([P, B * H], dt.float32, tag="den")
                    nc.vector.tensor_reduce(den[:], E2, Axis.X, Alu.add)
                    rec = sm_pool.tile([P, B * H], dt.float32, tag="rec")
                    nc.vector.reciprocal(rec[:], den[:])
                    A = sm_pool.tile([P, B, H, B], dt.float32, tag="A")
                    A2 = A[:].rearrange("p l h m -> p (l h) m")
                    nc.vector.tensor_tensor(
                        A2, E2, rec[:, :, None].to_broadcast([P, B * H, B]),
                        Alu.mult)

                    # combine: attn_out[l] = sum_m A[:,l,:,m] (bcast) * v[m]
                    for l in range(B):
                        acc = acc_pool.tile([P, D], dt.float32r, tag="acc")
                        nc.vector.tensor_tensor(
                            acc[:].rearrange("p (h d) -> p h d", d=HD),
                            v[0][:].rearrange("p (h d) -> p h d", d=HD),
                            A[:, l, :, 0, None].to_broadcast([P, H, HD]),
                            Alu.mult)
                        for m in range(1, B):
                            tmp = tt_pool.tile([P, D], dt.float32, tag="prod")
                            nc.vector.tensor_tensor(
                                tmp[:].rearrange("p (h d) -> p h d", d=HD),
                                v[m][:].rearrange("p (h d) -> p h d", d=HD),
                                A[:, l, :, m, None].to_broadcast([P, H, HD]),
                                Alu.mult)
                            nc.vector.tensor_tensor(acc[:], acc[:], tmp[:],
                                                    Alu.add)
                        row = l * n_loc + nt * P
                        nc.sync.dma_start(attn_d[row:row + P, :], acc[:])

                    # -- phi token-tiles for this n-slice (PE) --
                    for bb in range(B):
                        t = bb * NN + nt
                        for s in range(2):
                            ps = phi_psum.tile([P, 512], dt.float32, tag="phips")
                            for dtl in range(DT):
                                nc.tensor.matmul(
                                    ps[:], xT[dtl][:, t * P:(t + 1) * P],
                                    fmw[dtl][:, s * 512:(s + 1) * 512],
                                    start=(dtl == 0),
                                    stop=(not with_bias and dtl == DT - 1))
                            if with_bias:
                                nc.tensor.matmul(ps[:], ones_r[:, :P],
                                                 fmb[:, s * 512:(s + 1) * 512],
                                                 start=False, stop=True)
                            ev = phi_ev.tile([P, 512], dt.float32r, tag="phiev")
                            nc.scalar.copy(ev[:], ps[:])
                            nc.sync.dma_start(
                                phi_d[t * P:(t + 1) * P, s * 512:(s + 1) * 512],
                                ev[:])

                    # -- phiT column-slice ts=nt (PE) --
                    for pt in range(DT):
                        ps = phi_psum.tile([P, 512], dt.float32, tag="phiTps")
                        for dtl in range(DT):
                            nc.tensor.matmul(
                                ps[:], fmw[dtl][:, pt * P:(pt + 1) * P],
                                xT[dtl][:, nt * 512:(nt + 1) * 512],
                                start=(dtl == 0),
                                stop=(not with_bias and dtl == DT - 1))
                        if with_bias:
                            nc.tensor.matmul(ps[:], fmb[:, pt * P:(pt + 1) * P],
                                             ones_r[:], start=False, stop=True)
                        ev = phi_ev.tile([P, 512], dt.float32r, tag="phiTev")
                        nc.scalar.copy(ev[:], ps[:])
                        nc.sync.dma_start(
                            phiT_out[pt * P:(pt + 1) * P,
                                     nt * 512:(nt + 1) * 512], ev[:])

        # ---- Ph4: partial reduction over local tokens ----------------------
        # fast: red = 0.5 * attn^T @ phi  => P2'[din] (transposed P2 tiles)
        # bias: red = M = 0.5*((phi^T attn) @ outW^T + colsum(phi) x out_b)
        with tc.tile_pool(name="ow", bufs=DT) as ow_pool, \
             tc.tile_pool(name="ob", bufs=1) as ob_pool, \
             tc.tile_pool(name="chunks", bufs=NN + 2) as ch_pool, \
             tc.tile_pool(name="p2sb", bufs=DT) as p2_pool, \
             tc.tile_pool(name="sphi", bufs=2) as sphi_pool, \
             tc.tile_pool(name="p2ps", bufs=2, space="PSUM") as p2_psum, \
             tc.tile_pool(name="mps", bufs=2, space="PSUM") as m_psum, \
             tc.tile_pool(name="spps", bufs=2, space="PSUM") as sp_psum, \
             tc.tile_pool(name="mev", bufs=4) as mev_pool:
            if with_bias:
                ow = []
                for dtl in range(DT):
                    wt = ow_pool.tile([P, D], dt.float32r, tag="ow", name="ow")
                    nc.sync.dma_start(wt[:], out_wT[dtl * P:(dtl + 1) * P, :])
                    ow.append(wt)
                ob = ob_pool.tile([1, D], dt.float32r)
                nc.sync.dma_start(ob[:], out_b[:])

            for bb in range(B):
                ac = []; pc = []
                for c in range(NN):
                    row = bb * n_loc + c * P
                    a_t = ch_pool.tile([P, D], dt.float32r, tag="ach", name="ach")
                    p_t = ch_pool.tile([P, D], dt.float32r, tag="pch", name="pch")
                    nc.sync.dma_start(a_t[:], attn_d[row:row + P, :])
                    nc.sync.dma_start(p_t[:], phi_d[row:row + P, :])
                    ac.append(a_t); pc.append(p_t)

                if not with_bias:
                    # red_part[bb] = P2 = 0.5 * phi^T @ attn  (dphi, din)
                    for dtl in range(DT):
                        pps = p2_psum.tile([P, D], dt.float32, tag="p2ps",
                                           name="p2ps")
                        for c in range(NN):
                            for s in range(2):
                                nc.tensor.matmul(
                                    pps[:, s * 512:(s + 1) * 512],
                                    pc[c][:, dtl * P:(dtl + 1) * P],
                                    ac[c][:, s * 512:(s + 1) * 512],
                                    start=(c == 0), stop=(c == NN - 1))
                        ev = mev_pool.tile([P, D], dt.float32, tag="mev")
                        nc.scalar.mul(ev[:], pps[:], ALPHA)
                        nc.sync.dma_start(
                            red_part[bb, dtl * P:(dtl + 1) * P, :], ev[:])
                    continue

                # ---- general bias path: full M on device ----
                sp_ps = [sp_psum.tile([1, 512], dt.float32, tag="spps",
                                      name="spps") for _ in range(2)]
                for c in range(NN):
                    for s in range(2):
                        nc.tensor.matmul(sp_ps[s][:], ones_c[:],
                                         pc[c][:, s * 512:(s + 1) * 512],
                                         start=(c == 0), stop=(c == NN - 1))
                sphi = sphi_pool.tile([1, D], dt.float32r, tag="sphi")
                for s in range(2):
                    nc.vector.tensor_copy(sphi[:, s * 512:(s + 1) * 512],
                                          sp_ps[s][:])

                p2sb = []
                for dtl in range(DT):
                    pps = p2_psum.tile([P, D], dt.float32, tag="p2ps",
                                       name="p2ps")
                    for c in range(NN):
                        for s in range(2):
                            nc.tensor.matmul(
                                pps[:, s * 512:(s + 1) * 512],
                                ac[c][:, dtl * P:(dtl + 1) * P],
                                pc[c][:, s * 512:(s + 1) * 512],
                                start=(c == 0), stop=(c == NN - 1))
                    sb = p2_pool.tile([P, D], dt.float32r, tag="p2sb",
                                      name="p2sb")
                    nc.scalar.copy(sb[:], pps[:])
                    p2sb.append(sb)

                for half in range(2):
                    for pt in range(DT):
                        mps = m_psum.tile([P, 512], dt.float32, tag="mps")
                        for dtl in range(DT):
                            nc.tensor.matmul(
                                mps[:], p2sb[dtl][:, pt * P:(pt + 1) * P],
                                ow[dtl][:, half * 512:(half + 1) * 512],
                                start=(dtl == 0), stop=False)
                        nc.tensor.matmul(mps[:], sphi[:, pt * P:(pt + 1) * P],
                                         ob[:, half * 512:(half + 1) * 512],
                                         start=False, stop=True)
                        ev = mev_pool.tile([P, 512], dt.float32, tag="mevb")
                        nc.scalar.mul(ev[:], mps[:], ALPHA)
                        nc.sync.dma_start(
                            red_part[bb, pt * P:(pt + 1) * P,
                                     half * 512:(half + 1) * 512], ev[:])

    nc.compile()
    return nc


def build_launch2(n_loc: int, with_bias: bool):
    """Per-core program: final matmul chain + residual.

    fast:    y = x + (phi @ P2) @ out_w.T     (P2 = summed red_part)
    general: y = x + phi @ M                  (M  = summed red_part)
    """
    T = B * n_loc
    NN = n_loc // P
    DT = D // P

    nc = bacc.Bacc("TRN2", target_bir_lowering=False, debug=False,
                   num_devices=NCORES)

    phiT_in = nc.dram_tensor("phiT_in", [D, T], dt.float32r, kind="ExternalInput").ap()
    red = nc.dram_tensor("red", [B, D, D], dt.float32r, kind="ExternalInput").ap()
    x = nc.dram_tensor("x", [B, n_loc, D], dt.float32, kind="ExternalInput").ap()
    if not with_bias:
        out_wT = nc.dram_tensor("out_wT", [D, D], dt.float32r, kind="ExternalInput").ap()
    y = nc.dram_tensor("y", [T, D], dt.float32, kind="ExternalOutput").ap()

    xf = x.rearrange("b n d -> (b n) d")

    with tile.TileContext(nc) as tc, ExitStack() as ctx:
        phiT = None
        if with_bias:
            phiT_pool = ctx.enter_context(tc.tile_pool(name="phiT", bufs=DT))
            phiT = []
            for dtl in range(DT):
                t_ = phiT_pool.tile([P, T], dt.float32r, tag="phiT", name="phiT")
                nc.sync.dma_start(t_[:], phiT_in[dtl * P:(dtl + 1) * P, :])
                phiT.append(t_)

        if not with_bias:
            owp = ctx.enter_context(tc.tile_pool(name="owp", bufs=DT))
            ow = []
            for dtl in range(DT):
                wt = owp.tile([P, D], dt.float32r, tag="ow", name="ow")
                nc.gpsimd.dma_start(wt[:], out_wT[dtl * P:(dtl + 1) * P, :])
                ow.append(wt)

        with tc.tile_pool(name="mt", bufs=2 * DT) as m_pool, \
             tc.tile_pool(name="zt", bufs=DT + 2) as z_pool, \
             tc.tile_pool(name="phs", bufs=2 * DT) as ph_pool, \
             tc.tile_pool(name="xin", bufs=4) as x_pool, \
             tc.tile_pool(name="ysb", bufs=4) as y_pool, \
             tc.tile_pool(name="zps", bufs=3, space="PSUM") as z_psum, \
             tc.tile_pool(name="yps", bufs=2, space="PSUM") as y_psum:
            for bb in range(B):
                mt = []
                for dtl in range(DT):
                    t_ = m_pool.tile([P, D], dt.float32r, tag="mt", name="mt")
                    nc.sync.dma_start(t_[:], red[bb, dtl * P:(dtl + 1) * P, :])
                    mt.append(t_)

                if not with_bias:
                    # Z^T[din2, tok] = sum_dphi P2[dphi,din2] phiT[dphi,tok]
                    #   lhsT = red[bb] tiles (dphi part, din2 free)
                    phs = []
                    for dtl in range(DT):
                        ph_ = ph_pool.tile([P, n_loc], dt.float32r, tag="phs",
                                           name="phs")
                        nc.sync.dma_start(
                            ph_[:], phiT_in[dtl * P:(dtl + 1) * P,
                                            bb * n_loc:(bb + 1) * n_loc])
                        phs.append(ph_)
                    zt = []
                    for pt in range(DT):
                        zps = z_psum.tile([P, n_loc], dt.float32, tag="zps")
                        for dtl in range(DT):
                            nc.tensor.matmul(
                                zps[:], mt[dtl][:, pt * P:(pt + 1) * P],
                                phs[dtl][:],
                                start=(dtl == 0), stop=(dtl == DT - 1))
                        z_ = z_pool.tile([P, n_loc], dt.float32r, tag="zt",
                                         name="zt")
                        nc.scalar.copy(z_[:], zps[:])
                        zt.append(z_)
                    for c in range(NN):
                        tok = bb * n_loc + c * P
                        yps = y_psum.tile([P, D], dt.float32, tag="yps")
                        for pt in range(DT):
                            lhsT = zt[pt][:, c * P:(c + 1) * P]
                            for s in range(2):
                                nc.tensor.matmul(
                                    yps[:, s * 512:(s + 1) * 512], lhsT,
                                    ow[pt][:, s * 512:(s + 1) * 512],
                                    start=(pt == 0), stop=(pt == DT - 1))
                        xin = x_pool.tile([P, D], dt.float32, tag="xin")
                        nc.gpsimd.dma_start(xin[:], xf[tok:tok + P, :])
                        ysb = y_pool.tile([P, D], dt.float32, tag="ysb")
                        nc.vector.tensor_tensor(ysb[:], xin[:], yps[:], Alu.add)
                        nc.sync.dma_start(y[tok:tok + P, :], ysb[:])
                else:
                    for c in range(NN):
                        tok = bb * n_loc + c * P
                        yps = y_psum.tile([P, D], dt.float32, tag="yps")
                        for dtl in range(DT):
                            lhsT = phiT[dtl][:, tok:tok + P]
                            for s in range(2):
                                nc.tensor.matmul(
                                    yps[:, s * 512:(s + 1) * 512], lhsT,
                                    mt[dtl][:, s * 512:(s + 1) * 512],
                                    start=(dtl == 0), stop=(dtl == DT - 1))
                        xin = x_pool.tile([P, D], dt.float32, tag="xin")
                        nc.sync.dma_start(xin[:], xf[tok:tok + P, :])
                        ysb = y_pool.tile([P, D], dt.float32, tag="ysb")
                        nc.vector.tensor_tensor(ysb[:], xin[:], yps[:], Alu.add)
                        nc.sync.dma_start(y[tok:tok + P, :], ysb[:])

    nc.compile()
    return nc


_CACHE = {}


def _get_programs(n_loc: int, with_bias: bool):
    key = (n_loc, with_bias)
    if key not in _CACHE:
        _CACHE[key] = (build_launch1(n_loc, with_bias),
                       build_launch2(n_loc, with_bias))
    return _CACHE[key]


def kernel(x, fm_w, fm_b, in_proj_w, in_proj_b, out_w, out_b, _trace=False,
           _timings=None):
    x = np.ascontiguousarray(np.asarray(x, dtype=np.float32))
    Bx, N, Dx = x.shape
    assert (Bx, Dx) == (B, D) and N % NCORES == 0
    n_loc = N // NCORES

    fm_b_ = np.asarray(fm_b, np.float32).reshape(1, D)
    qkv_b_ = np.asarray(in_proj_b, np.float32).reshape(1, 3 * D)
    out_b_ = np.asarray(out_b, np.float32).reshape(1, D)
    with_bias = bool(fm_b_.any() or qkv_b_.any() or out_b_.any())

    nc1, nc2 = _get_programs(n_loc, with_bias)

    fm_wT = np.ascontiguousarray(np.asarray(fm_w, np.float32).T)
    wqkvT = np.ascontiguousarray(np.asarray(in_proj_w, np.float32).T)
    out_wT = np.ascontiguousarray(np.asarray(out_w, np.float32).T)

    x_shards = [np.ascontiguousarray(x[:, c * n_loc:(c + 1) * n_loc, :])
                for c in range(NCORES)]

    maps1 = [{
        "x": x_shards[c], "fm_wT": fm_wT, "fm_b": fm_b_, "wqkvT": wqkvT,
        "qkv_b": qkv_b_, "out_wT": out_wT, "out_b": out_b_,
    } for c in range(NCORES)]
    r1 = run_bass_kernel_spmd(nc1, maps1, core_ids=list(range(NCORES)),
                              trace=_trace)
    if _timings is not None:
        _timings.append(r1)

    red = np.zeros((B, D, D), np.float32)
    for c in range(NCORES):
        red += r1.results[c]["red_part"]

    maps2 = []
    for c in range(NCORES):
        m = {"phiT_in": r1.results[c]["phiT_out"], "red": red,
             "x": x_shards[c]}
        if not with_bias:
            m["out_wT"] = out_wT
        maps2.append(m)
    r2 = run_bass_kernel_spmd(nc2, maps2, core_ids=list(range(NCORES)),
                              trace=_trace)
    if _timings is not None:
        _timings.append(r2)

    out = np.concatenate(
        [r2.results[c]["y"].reshape(B, n_loc, D) for c in range(NCORES)],
        axis=1)
    return out

